# revision 9
# baseline (speedup 1.0000x reference)
"""Trainium2 Bass kernel for BuDingAttention (GQA attention layer).

Full inputs -> full output. Sharding: 8 cores = 2 batches x 4 KV heads.
Core c handles batch c//4, KV head c%4 (and its 4 query heads).
Each core computes q/k/v projections for its heads, RoPE, causal
attention, and the o_proj contribution of its heads (partial [S, H]).
Host sums the 4 partials per batch.
"""
import numpy as np

import concourse.bass as bass
import concourse.tile as tile
from concourse import bacc, mybir
from concourse.bass import ts, ds
from concourse.bass_utils import run_bass_kernel_spmd

B, S, H = 2, 2048, 2048
NH, NKV, HD = 16, 4, 128
G = NH // NKV          # query heads per KV head (= heads per core)
P = 128                # partitions
SB = S // P            # 16 row blocks
NG = SB // 4           # 4 groups of 4 row blocks
SC = S // 512          # 4 col chunks of 512
KC = H // P            # 16 contraction chunks for projections
ROPE_THETA = 10000.0
NEG = -1e9

f32 = mybir.dt.float32
f32r = mybir.dt.float32r

_CACHE = {}
LAST_RESULTS = None  # for test.py introspection


def _build(mode: str):
    """mode: 'causal' | 'nomask' | 'mask'"""
    nc = bacc.Bacc("TRN2", target_bir_lowering=False, debug=False,
                   enable_asserts=False, num_devices=8)

    xt_d = nc.dram_tensor("xt", [H, S], f32r, kind="ExternalInput").ap()
    wq_d = nc.dram_tensor("wq", [H, G * HD], f32r, kind="ExternalInput").ap()
    wk_d = nc.dram_tensor("wk", [H, HD], f32r, kind="ExternalInput").ap()
    wv_d = nc.dram_tensor("wv", [H, HD], f32r, kind="ExternalInput").ap()
    wo_d = nc.dram_tensor("wo", [G * HD, H], f32r, kind="ExternalInput").ap()
    bq_d = nc.dram_tensor("bq", [P, G], f32, kind="ExternalInput").ap()
    bk_d = nc.dram_tensor("bk", [P, 1], f32, kind="ExternalInput").ap()
    bv_d = nc.dram_tensor("bv", [P, 1], f32, kind="ExternalInput").ap()
    cost_d = nc.dram_tensor("cost", [P, S], f32, kind="ExternalInput").ap()
    sint_d = nc.dram_tensor("sint", [P, S], f32, kind="ExternalInput").ap()
    perm_d = nc.dram_tensor("perm", [P, P], f32r, kind="ExternalInput").ap()
    idr_d = nc.dram_tensor("idr", [P, P], f32, kind="ExternalInput").ap()
    tri_d = nc.dram_tensor("tri", [P, P], f32, kind="ExternalInput").ap()
    if mode == "mask":
        mask_d = nc.dram_tensor("mask", [S, S], f32, kind="ExternalInput").ap()
    out_d = nc.dram_tensor("out", [S, H], f32, kind="ExternalOutput").ap()

    xt_r = xt_d.rearrange("(ko p) s -> p ko s", p=P)       # [128,16,2048]
    wq_r = wq_d.rearrange("(ko p) m -> p ko m", p=P)       # [128,16,512]
    wk_r = wk_d.rearrange("(ko p) m -> p ko m", p=P)       # [128,16,128]
    wv_r = wv_d.rearrange("(ko p) m -> p ko m", p=P)
    wo_r = wo_d.rearrange("(ho p) n -> p ho n", p=P)       # [128,4,2048]

    with tile.TileContext(nc) as tc:
        with (
            tc.tile_pool(name="consts", bufs=1) as consts,
            tc.tile_pool(name="weights", bufs=1) as wpool,
            tc.tile_pool(name="acts", bufs=1) as apool,
        ):
            perm_t = consts.tile([P, P], f32r)
            idr_t = consts.tile([P, P], f32)
            tri_t = consts.tile([P, P], f32)
            bq_t = consts.tile([P, G], f32)
            bk_t = consts.tile([P, 1], f32)
            bv_t = consts.tile([P, 1], f32)
            nc.sync.dma_start(perm_t[:], perm_d)
            nc.sync.dma_start(idr_t[:], idr_d)
            nc.sync.dma_start(tri_t[:], tri_d)
            nc.sync.dma_start(bq_t[:], bq_d)
            nc.sync.dma_start(bk_t[:], bk_d)
            nc.sync.dma_start(bv_t[:], bv_d)

            wq_t = wpool.tile([P, KC, G * HD], f32r)
            wk_t = wpool.tile([P, KC, HD], f32r)
            wv_t = wpool.tile([P, KC, HD], f32r)
            wo_t = wpool.tile([P, G, H], f32r)
            nc.sync.dma_start(wq_t[:], wq_r)
            nc.sync.dma_start(wk_t[:], wk_r)
            nc.sync.dma_start(wv_t[:], wv_r)
            nc.sync.dma_start(wo_t[:], wo_r)

            qt_t = apool.tile([P, G, S], f32r)     # rotated Q^T per head
            kt_t = apool.tile([P, S], f32r)        # rotated K^T
            v_t = apool.tile([P, SB, HD], f32r)    # V rows, kpos on partition

            # ---------- Phase 1: QKV projections + RoPE ----------
            with (
                tc.tile_pool(name="ph1", bufs=1) as ph1,
                tc.tile_pool(name="ph1ps", bufs=1, space="PSUM") as ph1ps,
            ):
                cost_t = ph1.tile([P, S], f32)
                sint_t = ph1.tile([P, S], f32)
                nc.sync.dma_start(cost_t[:], cost_d)
                nc.sync.dma_start(sint_t[:], sint_d)

                for sc in range(SC):
                    scs = ds(sc * 512, 512)
                    q_ps = [ph1ps.tile([P, 512], f32, tag="qkv_ps", bufs=6,
                                       name=f"q_ps{i}")
                            for i in range(G)]
                    k_ps = ph1ps.tile([P, 512], f32, tag="qkv_ps", bufs=6)
                    v_ps = ph1ps.tile([P, 512], f32, tag="qkv_ps", bufs=6)
                    for kc4 in range(KC // 4):
                        xt_t = ph1.tile([P, 4, 512], f32r, tag="xt", bufs=2)
                        nc.sync.dma_start(
                            xt_t[:], xt_r[:, ts(kc4, 4), scs])
                        for ki in range(4):
                            kc = kc4 * 4 + ki
                            st, sp = kc == 0, kc == KC - 1
                            x_ap = xt_t[:, ki, :]
                            for blk in range(G):
                                nc.tensor.matmul(
                                    q_ps[blk][:],
                                    wq_t[:, kc, ts(blk, HD)], x_ap,
                                    start=st, stop=sp)
                            nc.tensor.matmul(k_ps[:], wk_t[:, kc, :], x_ap,
                                             start=st, stop=sp)
                            nc.tensor.matmul(v_ps[:], wv_t[:, kc, :], x_ap,
                                             start=st, stop=sp)

                    # rope for the 4 q blocks and k
                    for blk in range(G + 1):
                        src_ps = q_ps[blk] if blk < G else k_ps
                        bias = bq_t[:, blk:blk + 1] if blk < G else bk_t[:, :]
                        raw = ph1.tile([P, 512], f32r, tag="qkraw", bufs=3)
                        nc.scalar.activation(
                            raw[:], src_ps[:],
                            mybir.ActivationFunctionType.Identity,
                            bias=bias)
                        rot_ps = ph1ps.tile([P, 512], f32, tag="mps", bufs=2)
                        nc.tensor.matmul(rot_ps[:], perm_t[:], raw[:],
                                         start=True, stop=True)
                        tmp_sin = ph1.tile([P, 512], f32, tag="rtmp", bufs=4)
                        nc.vector.tensor_tensor(
                            tmp_sin[:], rot_ps[:], sint_t[:, scs],
                            mybir.AluOpType.mult)
                        tmp_cos = ph1.tile([P, 512], f32, tag="rtmp", bufs=4)
                        nc.vector.tensor_tensor(
                            tmp_cos[:], raw[:], cost_t[:, scs],
                            mybir.AluOpType.mult)
                        dst = (qt_t[:, blk, scs] if blk < G
                               else kt_t[:, scs])
                        nc.vector.tensor_tensor(dst, tmp_cos[:], tmp_sin[:],
                                                mybir.AluOpType.add)

                    # V: bias copyback then PE transpose into [s, hd]
                    vt_sb = ph1.tile([P, 512], f32, tag="vt", bufs=2)
                    nc.scalar.activation(
                        vt_sb[:], v_ps[:],
                        mybir.ActivationFunctionType.Identity,
                        bias=bv_t[:, :])
                    for j in range(4):
                        vtr_ps = ph1ps.tile([P, P], f32, tag="mps", bufs=2)
                        nc.tensor.transpose(vtr_ps[:], vt_sb[:, ts(j, P)],
                                            idr_t[:])
                        nc.vector.tensor_copy(v_t[:, sc * 4 + j, :],
                                              vtr_ps[:])

            # ---------- Phase 2: attention + o_proj ----------
            with (
                tc.tile_pool(name="ph2", bufs=1) as ph2,
                tc.tile_pool(name="ph2ps", bufs=1, space="PSUM") as ph2ps,
            ):
                for g in range(NG):
                    nkp = 4 * g + 4     # kpos chunks valid in this group
                    ot_sb = ph2.tile([P, G, 512], f32r, tag="OT", bufs=2)
                    for h in range(G):
                        p_rows = []
                        for j in range(4):
                            r = 4 * g + j
                            L = (r + 1) * P
                            nch = (r // 4) + 1   # 512-wide chunks
                            p_t = ph2.tile([P, S], f32, tag="P", bufs=5)
                            sums = ph2.tile([P, 4], f32, tag="sums", bufs=4)
                            for c in range(nch):
                                w = min(512, L - 512 * c)
                                s_ps = ph2ps.tile([P, 512], f32,
                                                  tag="sc_ps", bufs=2)
                                nc.tensor.matmul(
                                    s_ps[:, :w],
                                    qt_t[:, h, ts(r, P)],
                                    kt_t[:, ds(512 * c, w)],
                                    start=True, stop=True)
                                if mode == "mask":
                                    mrow = mask_d.rearrange(
                                        "(rb p) s -> rb p s", p=P)
                                    m_sb = ph2.tile([P, 512], f32,
                                                    tag="msk", bufs=3)
                                    nc.sync.dma_start(
                                        m_sb[:, :w],
                                        mrow[r, :, ds(512 * c, w)])
                                    nc.vector.tensor_tensor(
                                        s_ps[:, :w], s_ps[:, :w],
                                        m_sb[:, :w], mybir.AluOpType.add)
                                if mode == "causal" and c == nch - 1:
                                    # diagonal block: additive -1e9 upper tri
                                    nc.vector.tensor_tensor(
                                        s_ps[:, w - P:w], s_ps[:, w - P:w],
                                        tri_t[:], mybir.AluOpType.add)
                                nc.scalar.activation(
                                    p_t[:, ds(512 * c, w)], s_ps[:, :w],
                                    mybir.ActivationFunctionType.Exp,
                                    accum_out=sums[:, c:c + 1])
                            if nch > 1:
                                lsum = ph2.tile([P, 1], f32, tag="lsum",
                                                bufs=4)
                                nc.vector.tensor_reduce(
                                    lsum[:], sums[:, :nch],
                                    mybir.AxisListType.X, mybir.AluOpType.add)
                                lsrc = lsum
                            else:
                                lsrc = sums
                            rcp = ph2.tile([P, 1], f32, tag="rcp", bufs=4)
                            nc.vector.reciprocal(rcp[:], lsrc[:, 0:1])
                            nc.vector.tensor_scalar_mul(
                                p_t[:, :L], p_t[:, :L], rcp[:, :])
                            p_rows.append(p_t)

                        ot_ps = ph2ps.tile([P, 512], f32, tag="ot_ps", bufs=2)
                        for k in range(nkp):
                            j0 = max(0, k - 4 * g)
                            pt_t = ph2.tile([P, 512], f32r, tag="PT", bufs=2)
                            for j in range(j0, 4):
                                ptr_ps = ph2ps.tile([P, P], f32,
                                                    tag="pt_ps", bufs=2)
                                nc.tensor.transpose(
                                    ptr_ps[:],
                                    p_rows[j][:, ts(k, P)], idr_t[:])
                                nc.vector.tensor_copy(pt_t[:, ts(j, P)],
                                                      ptr_ps[:])
                            nc.tensor.matmul(
                                ot_ps[:, ds(j0 * P, 512 - j0 * P)],
                                v_t[:, k, :],
                                pt_t[:, ds(j0 * P, 512 - j0 * P)],
                                start=(k == 0), stop=(k == nkp - 1))
                        nc.vector.tensor_copy(ot_sb[:, h, :], ot_ps[:])

                    # o_proj for this group's 4 row blocks
                    for j in range(4):
                        r = 4 * g + j
                        for hc in range(SC):
                            o_ps = ph2ps.tile([P, 512], f32,
                                              tag="op_ps", bufs=2)
                            for h in range(G):
                                nc.tensor.matmul(
                                    o_ps[:],
                                    ot_sb[:, h, ts(j, P)],
                                    wo_t[:, h, ts(hc, 512)],
                                    start=(h == 0), stop=(h == G - 1))
                            o_sb = ph2.tile([P, 512], f32, tag="osb", bufs=3)
                            nc.scalar.copy(o_sb[:], o_ps[:])
                            nc.sync.dma_start(
                                out_d[ts(r, P), ts(hc, 512)], o_sb[:])

    nc.compile()
    return nc


def _get_program(mode: str):
    if mode not in _CACHE:
        _CACHE[mode] = _build(mode)
    return _CACHE[mode]


def _detect_mode(attention_mask: np.ndarray) -> str:
    m = attention_mask[:, 0]  # [B, S, S]
    if not np.isfinite(m).all():
        return "mask"
    if np.abs(m).max() == 0.0:
        return "nomask"
    iu = np.triu_indices(S, k=1)
    il = np.tril_indices(S, k=0)
    for b in range(m.shape[0]):
        if not (np.all(m[b][iu] <= -1e8) and np.all(m[b][il] == 0.0)):
            return "mask"
    return "causal"


def _rope_tables(position_ids: np.ndarray):
    """cos/sin transposed to [HD, S] per batch."""
    inv_freq = 1.0 / (ROPE_THETA **
                      (np.arange(0, HD, 2, dtype=np.float64) / HD))
    out = []
    for b in range(position_ids.shape[0]):
        freqs = position_ids[b].astype(np.float64)[:, None] * inv_freq
        emb = np.concatenate([freqs, freqs], axis=-1)  # [S, HD]
        cost = np.ascontiguousarray(np.cos(emb).T.astype(np.float32))
        sint = np.ascontiguousarray(np.sin(emb).T.astype(np.float32))
        out.append((cost, sint))
    return out


def kernel(hidden_states, wq, bq, wk, bk, wv, bv, wo,
           attention_mask, position_ids, _profile=False, _trace_cores=None):
    global LAST_RESULTS
    hidden_states = np.asarray(hidden_states, dtype=np.float32)
    wq = np.asarray(wq, dtype=np.float32)
    bq = np.asarray(bq, dtype=np.float32)
    wk = np.asarray(wk, dtype=np.float32)
    bk = np.asarray(bk, dtype=np.float32)
    wv = np.asarray(wv, dtype=np.float32)
    bv = np.asarray(bv, dtype=np.float32)
    wo = np.asarray(wo, dtype=np.float32)
    attention_mask = np.asarray(attention_mask, dtype=np.float32)
    position_ids = np.asarray(position_ids)

    mode = _detect_mode(attention_mask)
    nc = _get_program(mode)

    rope = _rope_tables(position_ids)

    half = HD // 2
    # lhsT for rot = Pi @ q: matmul computes lhsT.T @ rhs, Pi=[[0,-I],[I,0]]
    perm = np.zeros((P, P), dtype=np.float32)
    perm[0:half, half:P] = np.eye(half, dtype=np.float32)
    perm[half:P, 0:half] = -np.eye(half, dtype=np.float32)
    ident = np.eye(P, dtype=np.float32)
    tri = np.where(np.tril(np.ones((P, P), dtype=bool)), 0.0, NEG)
    tri = tri.astype(np.float32)

    scale = 1.0 / np.sqrt(HD)
    in_maps = []
    for c in range(8):
        b, kv = c // NKV, c % NKV
        cost, sint = rope[b]
        im = {
            "xt": np.ascontiguousarray(hidden_states[b].T),
            "wq": np.ascontiguousarray(wq[:, 512 * kv:512 * (kv + 1)]) * scale,
            "wk": np.ascontiguousarray(wk[:, HD * kv:HD * (kv + 1)]),
            "wv": np.ascontiguousarray(wv[:, HD * kv:HD * (kv + 1)]),
            "wo": np.ascontiguousarray(wo[512 * kv:512 * (kv + 1), :]),
            "bq": np.ascontiguousarray(
                bq[512 * kv:512 * (kv + 1)].reshape(G, HD).T) * scale,
            "bk": bk[HD * kv:HD * (kv + 1)].reshape(HD, 1).copy(),
            "bv": bv[HD * kv:HD * (kv + 1)].reshape(HD, 1).copy(),
            "cost": cost, "sint": sint,
            "perm": perm, "idr": ident, "tri": tri,
        }
        im = {k: np.ascontiguousarray(v, dtype=np.float32)
              for k, v in im.items()}
        if mode == "mask":
            im["mask"] = np.ascontiguousarray(attention_mask[b, 0],
                                              dtype=np.float32)
        in_maps.append(im)

    kwargs = {}
    if _profile:
        kwargs = dict(trace=True,
                      trace_cores=_trace_cores or [0])
    res = run_bass_kernel_spmd(nc, in_maps, core_ids=list(range(8)), **kwargs)
    LAST_RESULTS = res

    out = np.zeros((B, S, H), dtype=np.float32)
    for c in range(8):
        out[c // NKV] += res.results[c]["out"]
    return out
